# revision 1
# baseline (speedup 1.0000x reference)
"""Causal self-attention (B=4, S=2048, D=1024, H=16, rope) on 8 trn2 cores.

Sharding: batch x head-half. Core c handles batch b=c//2 and heads
hh*8..hh*8+7 where hh=c%2. Each core computes its 8 heads' attention over its
batch and a partial output projection; the host sums the two partials per
batch.

All heavy matmuls run in fp32r (TF32-like, 11-bit mantissa, 4x faster than
fp32 on the PE at moving-dim >= 256). End-to-end absmax error vs fp32
reference ~2.5e-4 of output scale (validated by numpy simulation of the
rounding).
"""

import numpy as np

B, S, D, H, DK = 4, 2048, 1024, 16, 64
THETA = 10000.0
N_CORES = 8
HPC = H // 2          # heads per core
OC = 4                # 128-row output chunks per core (512 cols of D)
SC4 = 4               # 512-wide s chunks
NKT = S // 128        # k tiles
F32R_ROUND = True

_prog_cache = {}


def _apply_walrus_wait_workarounds():
    """This container's walrus rejects any TPB instruction with more than one
    sync wait. Patch the Tile kernel-tail drain to emit a chain of single-wait
    drains, and provide a post-pass that hoists excess waits onto NoOps."""
    import concourse.mybir as mybir
    import concourse.tile as tile_mod
    from concourse.vector_clock import ScopedClock

    def _drain_and_barrier(self, tick_clock, wait_clock):
        nc = self.nc
        drain_inst = nc.sync.drain()
        wait_clock.add_sem_waits(
            drain_inst.ins, ScopedClock({None: tick_clock.global_clock}))
        waits = list(drain_inst.ins.sync_info.on_wait)
        if len(waits) > 1:
            si = drain_inst.ins.sync_info
            si.on_wait = waits[:1]
            drain_inst.ins.sync_info = si
            for i in range(1, len(waits)):
                d2 = nc.sync.drain()
                d2.ins.sync_info = mybir.SyncInfo(
                    on_wait=waits[i:i + 1], on_update=[])
        nc.all_engine_barrier()
        popped = nc._tile_sem_poison_stack.pop()
        assert popped is self._sem_poison
        nc.clear_and_free_semaphores(list(self.sems.allocated().values()))
        nc.all_engine_barrier()

    tile_mod.TileContext._drain_and_barrier = _drain_and_barrier


def _split_waits(nc):
    import concourse.mybir as mybir
    engines = {mybir.EngineType.PE, mybir.EngineType.DVE, mybir.EngineType.SP,
               mybir.EngineType.Activation, mybir.EngineType.Pool}
    for f in nc.m.functions:
        for bb in f.blocks:
            out = []
            changed = False
            for ins in bb.instructions:
                si = ins.sync_info
                if si is not None and len(si.on_wait) > 1 and ins.engine in engines:
                    waits = list(si.on_wait)
                    for i in range(len(waits) - 1):
                        out.append(mybir.InstNoOp(
                            name=f"{ins.name}-waitsplit-{i}",
                            sync_info=mybir.SyncInfo(
                                on_wait=waits[i:i + 1], on_update=[]),
                            bass_nofuse=True, engine=ins.engine))
                    ins.sync_info = mybir.SyncInfo(
                        on_wait=waits[-1:], on_update=list(si.on_update))
                    changed = True
                out.append(ins)
            if changed:
                bb.instructions = out


def _build_program():
    _apply_walrus_wait_workarounds()
    import concourse.bass as bass
    import concourse.mybir as mybir
    import concourse.tile as tile
    from concourse.masks import make_identity
    from contextlib import ExitStack

    F32 = mybir.dt.float32
    F32R = mybir.dt.float32r
    AF = mybir.ActivationFunctionType

    nc = bass.Bass()
    xb = nc.declare_dram_parameter("xb", [S, D], F32, isOutput=False)
    wqt = nc.declare_dram_parameter("wqt", [D, 512], F32, isOutput=False)
    wkt = nc.declare_dram_parameter("wkt", [D, 512], F32, isOutput=False)
    wvt = nc.declare_dram_parameter("wvt", [D, 512], F32, isOutput=False)
    wot = nc.declare_dram_parameter("wot", [512, D], F32, isOutput=False)
    cost = nc.declare_dram_parameter("cost", [128, S], F32, isOutput=False)
    sint2 = nc.declare_dram_parameter("sint2", [128, S], F32, isOutput=False)
    esel = nc.declare_dram_parameter("esel", [32, 16, 128], F32, isOutput=False)
    y = nc.declare_dram_parameter("y", [S, D], F32, isOutput=True)

    with tile.TileContext(nc) as tc, ExitStack() as ctx:
        singles = ctx.enter_context(tc.tile_pool(name="singles", bufs=1))
        ident = singles.tile([128, 128], F32)
        make_identity(nc, ident)

        # persistent slabs
        qslab = singles.tile([128, OC, S], F32R, tag="qslab")   # doubles as attn_outT
        kslab = singles.tile([128, OC, S], F32R, tag="kslab")
        vslab = singles.tile([128, NKT, HPC, 65], F32R, tag="vslab")
        # ones column of the v slab: memset f32 staging then rounded copy
        ones_col = singles.tile([128, NKT, HPC, 1], F32, tag="ones_col")
        nc.vector.memset(ones_col, 1.0)
        nc.vector.tensor_copy(vslab[:, :, :, 64:65], ones_col)

        # ---------------- phase 1: transpose x, project q/k/v, rope ----------
        with tc.tile_pool(name="ph1", bufs=1) as ph1, \
             tc.tile_pool(name="wstage", bufs=2) as wstage, \
             tc.tile_pool(name="xpool", bufs=4) as xpool, \
             tc.tile_pool(name="xtpool", bufs=2) as xtpool, \
             tc.tile_pool(name="ropetmp", bufs=2) as ropetmp, \
             tc.tile_pool(name="pstr", bufs=4, space="PSUM") as pstr, \
             tc.tile_pool(name="psp", bufs=4, space="PSUM") as psp:

            x_prefetch = []
            for ssub in range(4):
                xt = xpool.tile([128, D], F32, tag="x", name=f"xpre{ssub}")
                nc.sync.dma_start(out=xt, in_=xb[ssub * 128:(ssub + 1) * 128, :])
                x_prefetch.append(xt)

            # round weights to f32r via ACT copies, in [128, 512] chunks
            wr = {}
            for name, src in (("q", wqt), ("k", wkt), ("v", wvt)):
                wr[name] = ph1.tile([128, 8, 512], F32R, tag=f"w{name}r",
                                    name=f"w{name}r")
                src_r = src.rearrange("(ic p) o -> p ic o", p=128)
                for ic in range(8):
                    st = wstage.tile([128, 512], F32, tag="wstage")
                    nc.sync.dma_start(out=st, in_=src_r[:, ic, :])
                    nc.scalar.copy(out=wr[name][:, ic, :], in_=st)

            for sc4 in range(SC4):
                ssl = slice(sc4 * 512, (sc4 + 1) * 512)
                cosc = ropetmp.tile([128, 512], F32, tag="cosc", bufs=1)
                nc.sync.dma_start(out=cosc, in_=cost[:, ssl])
                sinc = ropetmp.tile([128, 512], F32, tag="sinc", bufs=1)
                nc.sync.dma_start(out=sinc, in_=sint2[:, ssl])
                xtc = xtpool.tile([128, 8, 512], F32R, tag="xtc")
                if sc4 == 0:
                    xts = x_prefetch
                else:
                    xts = []
                    for ssub in range(4):
                        xt = xpool.tile([128, D], F32, tag="x")
                        s0 = sc4 * 512 + ssub * 128
                        nc.sync.dma_start(out=xt, in_=xb[s0:s0 + 128, :])
                        xts.append(xt)
                for ic in range(8):
                    ptr = pstr.tile([128, 512], F32, tag="ptr")
                    for ssub in range(4):
                        nc.tensor.transpose(
                            ptr[:, ssub * 128:(ssub + 1) * 128],
                            xts[ssub][:, ic * 128:(ic + 1) * 128], ident)
                    nc.scalar.copy(out=xtc[:, ic, :], in_=ptr)

                # q/k projections with rope
                for wname, slab in (("q", qslab), ("k", kslab)):
                    for oc in range(OC):
                        pp = psp.tile([128, 512], F32, tag="pp")
                        for ic in range(8):
                            nc.tensor.matmul(
                                pp, lhsT=wr[wname][:, ic, oc * 128:(oc + 1) * 128],
                                rhs=xtc[:, ic, :],
                                start=(ic == 0), stop=(ic == 7))
                        tsh = ropetmp.tile([128, 512], F32, tag="tsh")
                        nc.vector.stream_shuffle(tsh, pp, _pair_swap_mask())
                        nc.vector.tensor_mul(slab[:, oc, ssl], pp, cosc)
                        nc.gpsimd.tensor_mul(tsh, tsh, sinc)
                        nc.vector.tensor_add(slab[:, oc, ssl], slab[:, oc, ssl], tsh)

                # v projection (natural [s, o] layout)
                for ssub in range(4):
                    pv = psp.tile([128, 512], F32, tag="pp")
                    for ic in range(8):
                        nc.tensor.matmul(
                            pv, lhsT=xtc[:, ic, ssub * 128:(ssub + 1) * 128],
                            rhs=wr["v"][:, ic, :],
                            start=(ic == 0), stop=(ic == 7))
                    kt = sc4 * 4 + ssub
                    nc.scalar.copy(
                        out=vslab[:, kt, :, 0:64],
                        in_=pv.rearrange("p (h dk) -> p h dk", h=HPC))

        # ---------------- phase 2: attention ---------------------------------
        with tc.tile_pool(name="norm", bufs=1) as norm:
            # prefetch + round the output-projection weights here so they
            # are ready long before phase 4
            wor = norm.tile([128, 4, D], F32R, tag="wor")
            wot_r = wot.rearrange("(ic p) o -> p ic o", p=128)
            for ic in range(4):
                wst = norm.tile([128, D], F32, tag="wostage", name=f"wst{ic}")
                nc.sync.dma_start(out=wst, in_=wot_r[:, ic, :])
                nc.scalar.copy(out=wor[:, ic, :], in_=wst)
            # row h*4 + c holds the softmax denominators of head h, q-chunk c
            sums = norm.tile([32, 512], F32, tag="sums")
            recips = norm.tile([32, 512], F32, tag="recips")
            esl = norm.tile([32, 16, 128], F32, tag="esl")
            nc.sync.dma_start(out=esl, in_=esel[:])

            with tc.tile_pool(name="ptpool", bufs=4) as ptpool, \
                 tc.tile_pool(name="stmp", bufs=2) as stmpp, \
                 tc.tile_pool(name="pss", bufs=2, space="PSUM") as pss, \
                 tc.tile_pool(name="pso", bufs=4, space="PSUM") as pso:
                for h in range(HPC):
                    r0 = (h % 2) * 64
                    oc = h // 2
                    pos = {}
                    for c in range(4):
                        pos[c] = pso.tile([65, 512], F32, tag="po", name=f"po{c}")
                    # j-outer in blocks of 4: the k/v stationary of each j is
                    # reused across all valid q-chunks (LDWEIGHTS amortized), and
                    # the per-block QK->exp->PV grouping bounds live pt tiles
                    for blk in range(4):
                        pts = {}
                        for j in range(4 * blk, 4 * blk + 4):
                            cmin = j // 4
                            cs = list(range(cmin, 4))
                            for ci in range(0, len(cs), 2):
                                cpair = cs[ci:ci + 2]
                                w = len(cpair) * 512
                                ps = pss.tile([128, 1024], F32, tag="ps", name="ps")
                                for idx, c in enumerate(cpair):
                                    d = max(0, j * 128 - c * 512)
                                    nc.tensor.matmul(
                                        ps[:, idx * 512 + d:(idx + 1) * 512],
                                        lhsT=kslab[r0:r0 + 64, oc, j * 128:(j + 1) * 128],
                                        rhs=qslab[r0:r0 + 64, oc, c * 512 + d:(c + 1) * 512],
                                        start=True, stop=True)
                                pt = ptpool.tile([128, 1024], F32R, tag="pt")
                                nc.scalar.activation(out=pt[:, 0:w], in_=ps[:, 0:w],
                                                     func=AF.Exp, scale=0.125)
                                d = j * 128 - cpair[0] * 512
                                if 0 <= d:  # first chunk of the pair is diagonal
                                    nc.gpsimd.affine_select(
                                        out=pt[:, d:d + 128], in_=pt[:, d:d + 128],
                                        compare_op=mybir.AluOpType.is_ge,
                                        fill=0.0, base=0,
                                        pattern=[[1, 128]], channel_multiplier=-1)
                                pts[(j, ci)] = pt
                        for j in range(4 * blk, 4 * blk + 4):
                            cmin = j // 4
                            cs = list(range(cmin, 4))
                            for ci in range(0, len(cs), 2):
                                pt = pts[(j, ci)]
                                for idx, c in enumerate(cs[ci:ci + 2]):
                                    d = max(0, j * 128 - c * 512)
                                    nc.tensor.matmul(
                                        pos[c][:, d:512],
                                        lhsT=vslab[:, j, h, 0:65],
                                        rhs=pt[:, idx * 512 + d:(idx + 1) * 512],
                                        start=(j == 0), stop=(j == 4 * c + 3))
                        # q-chunk `blk` got its last contribution in this block:
                        # write back its unnormalized output + denominators
                        c = blk
                        qsl = slice(c * 512, (c + 1) * 512)
                        po = pos[c]
                        nc.vector.tensor_copy(qslab[r0:r0 + 64, oc, qsl], po[0:64, :])
                        stmp = stmpp.tile([1, 512], mybir.dt.float32, tag="stmp")
                        nc.vector.tensor_copy(stmp, po[64:65, :])
                        hc = h * 4 + c
                        nc.sync.dma_start(out=sums[hc:hc + 1, :], in_=stmp)

            # normalize: recip of all denominators, broadcast via selector
                # matmuls, scale the attention outputs in place
                nc.vector.reciprocal(recips, sums)
                for a in range(OC):
                    for c in range(4):
                        pb = pso.tile([128, 512], F32, tag="po", name="pb")
                        nc.tensor.matmul(pb, lhsT=esl[:, a * 4 + c, :],
                                         rhs=recips,
                                         start=True, stop=True)
                        qsl = slice(c * 512, (c + 1) * 512)
                        nc.vector.tensor_mul(qslab[:, a, qsl], qslab[:, a, qsl], pb)

            # ------------- phase 4: output projection (attention PSUM pools
            # closed so psy can take their banks; norm holds wor) -------------
            with tc.tile_pool(name="ysb", bufs=3) as ysb, \
                 tc.tile_pool(name="psy", bufs=4, space="PSUM") as psy:
                for qs in range(16):
                    yt = ysb.tile([128, D], F32, tag="yt")
                    for oh in range(2):
                        py = psy.tile([128, 512], F32, tag="py")
                        for ic in range(4):
                            nc.tensor.matmul(
                                py, lhsT=qslab[:, ic, qs * 128:(qs + 1) * 128],
                                rhs=wor[:, ic, oh * 512:(oh + 1) * 512],
                                start=(ic == 0), stop=(ic == 3))
                        nc.scalar.copy(out=yt[:, oh * 512:(oh + 1) * 512], in_=py)
                    nc.sync.dma_start(out=y[qs * 128:(qs + 1) * 128, :], in_=yt)


    _split_waits(nc)
    return nc


def _pair_swap_mask():
    mask = []
    for j in range(16):
        mask += [2 * j + 1, 2 * j]
    return mask


def _host_inputs(x, wq, wk, wv, wo, token_positions):
    pos = np.asarray(token_positions).astype(np.float64)
    ex = np.arange(0, DK, 2, dtype=np.float64) / DK
    freq = 1.0 / (THETA ** ex)
    f = pos[:, None] * freq[None, :]                       # [S, DK/2]
    cos = np.repeat(np.cos(f), 2, axis=1).astype(np.float32)   # [S, DK]
    sin = np.repeat(np.sin(f), 2, axis=1).astype(np.float32)
    cosT = np.ascontiguousarray(cos.T)                     # [DK, S]
    sinT = np.ascontiguousarray(sin.T)
    sgn = np.where(np.arange(DK) % 2 == 0, -1.0, 1.0).astype(np.float32)
    sinT2 = sinT * sgn[:, None]
    cost = np.tile(cosT, (2, 1))                           # [128, S]
    sint2 = np.tile(sinT2, (2, 1))

    # selector matrices for the sum-broadcast matmul: within a head-pair's
    # 32-row sums block, row (m>=64)*4 + c holds the denominators for
    # output partition m, q-chunk c
    esel = np.zeros((32, 16, 128), np.float32)
    for a in range(4):
        for c in range(4):
            esel[8 * a + c, a * 4 + c, 0:64] = 1.0
            esel[8 * a + 4 + c, a * 4 + c, 64:128] = 1.0

    wqT = np.ascontiguousarray(wq.T)
    wkT = np.ascontiguousarray(wk.T)
    wvT = np.ascontiguousarray(wv.T)
    woT = np.ascontiguousarray(wo.T)

    in_maps = []
    for core in range(N_CORES):
        b, hh = core // 2, core % 2
        osl = slice(hh * 512, (hh + 1) * 512)
        in_maps.append({
            "xb": np.ascontiguousarray(x[b]),
            "wqt": np.ascontiguousarray(wqT[:, osl]),
            "wkt": np.ascontiguousarray(wkT[:, osl]),
            "wvt": np.ascontiguousarray(wvT[:, osl]),
            "wot": np.ascontiguousarray(woT[osl, :]),
            "cost": cost,
            "sint2": sint2,
            "esel": esel,
        })
    return in_maps


def run_sharded(x, wq, wk, wv, wo, token_positions, trace=False):
    from concourse.bass_utils import run_bass_kernel_spmd
    if "nc" not in _prog_cache:
        _prog_cache["nc"] = _build_program()
    nc = _prog_cache["nc"]
    in_maps = _host_inputs(x, wq, wk, wv, wo, token_positions)
    res = run_bass_kernel_spmd(nc, in_maps, list(range(N_CORES)), trace=trace)
    out = np.empty((B, S, D), np.float32)
    for b in range(B):
        out[b] = res.results[2 * b]["y"] + res.results[2 * b + 1]["y"]
    return out, res


def kernel(x, wq, wk, wv, wo, token_positions):
    x = np.asarray(x, dtype=np.float32)
    out, _ = run_sharded(
        x, np.asarray(wq, np.float32), np.asarray(wk, np.float32),
        np.asarray(wv, np.float32), np.asarray(wo, np.float32),
        np.asarray(token_positions))
    return out



# revision 18
# speedup vs baseline: 1.2844x; 1.2844x over previous
"""Causal self-attention (B=4, S=2048, D=1024, H=16, rope) on 8 trn2 cores.

Sharding: batch x head-half. Core c handles batch b=c//2 and heads
hh*8..hh*8+7 where hh=c%2. Each core computes its 8 heads' attention over its
batch and a partial output projection; the host sums the two partials per
batch.

v2: fully pipelined head-pair schedule. The 8 local heads form 4 pairs
(pair p = local dims [128p, 128p+128)). Per pair: q/k projection (bf16
inputs, f32r slabs), then attention over 4 q-quarters of 512 with the two
heads' QK^T matmuls packed onto disjoint PE row halves (tile_position via
base partitions 0/64), one exp per (quarter, k-tile) covering both heads,
PV in bf16 with a ones-column producing softmax denominators, and per-
quarter normalization via reciprocal_approx_fast + gpsimd
partition_broadcast. Projection of pair p+1 is interleaved into pair p's
attention so the PE stays busy while ACT streams exps; the output
projection is interleaved into pair 3's attention.
"""

import numpy as np

B, S, D, H, DK = 4, 2048, 1024, 16, 64
THETA = 10000.0
N_CORES = 8
NKT = S // 128        # k tiles
NP = 4                # head pairs per core
NQ = 4                # q quarters of 512

_prog_cache = {}


def _apply_walrus_wait_workarounds():
    """This container's walrus rejects any TPB instruction with more than one
    sync wait. Patch the Tile kernel-tail drain to emit a chain of single-wait
    drains, and provide a post-pass that hoists excess waits onto NoOps."""
    import concourse.mybir as mybir
    import concourse.tile as tile_mod
    from concourse.vector_clock import ScopedClock

    def _drain_and_barrier(self, tick_clock, wait_clock):
        nc = self.nc
        drain_inst = nc.sync.drain()
        wait_clock.add_sem_waits(
            drain_inst.ins, ScopedClock({None: tick_clock.global_clock}))
        waits = list(drain_inst.ins.sync_info.on_wait)
        if len(waits) > 1:
            si = drain_inst.ins.sync_info
            si.on_wait = waits[:1]
            drain_inst.ins.sync_info = si
            for i in range(1, len(waits)):
                d2 = nc.sync.drain()
                d2.ins.sync_info = mybir.SyncInfo(
                    on_wait=waits[i:i + 1], on_update=[])
        nc.all_engine_barrier()
        popped = nc._tile_sem_poison_stack.pop()
        assert popped is self._sem_poison
        nc.clear_and_free_semaphores(list(self.sems.allocated().values()))
        nc.all_engine_barrier()

    tile_mod.TileContext._drain_and_barrier = _drain_and_barrier


def _split_waits(nc):
    import concourse.mybir as mybir
    engines = {mybir.EngineType.PE, mybir.EngineType.DVE, mybir.EngineType.SP,
               mybir.EngineType.Activation, mybir.EngineType.Pool}
    for f in nc.m.functions:
        for bb in f.blocks:
            out = []
            changed = False
            for ins in bb.instructions:
                si = ins.sync_info
                if si is not None and len(si.on_wait) > 1 and ins.engine in engines:
                    waits = list(si.on_wait)
                    for i in range(len(waits) - 1):
                        out.append(mybir.InstNoOp(
                            name=f"{ins.name}-waitsplit-{i}",
                            sync_info=mybir.SyncInfo(
                                on_wait=waits[i:i + 1], on_update=[]),
                            bass_nofuse=True, engine=ins.engine))
                    ins.sync_info = mybir.SyncInfo(
                        on_wait=waits[-1:], on_update=list(si.on_update))
                    changed = True
                out.append(ins)
            if changed:
                bb.instructions = out


def _pair_swap_mask():
    mask = []
    for j in range(16):
        mask += [2 * j + 1, 2 * j]
    return mask


def _build_program():
    _apply_walrus_wait_workarounds()
    import concourse.bass as bass
    import concourse.mybir as mybir
    import concourse.tile as tile
    from concourse import library_config
    from concourse.masks import make_identity
    from contextlib import ExitStack

    F32 = mybir.dt.float32
    F32R = mybir.dt.float32r
    BF16 = mybir.dt.bfloat16
    AF = mybir.ActivationFunctionType

    nc = bass.Bass()
    xb = nc.declare_dram_parameter("xb", [S, D], F32, isOutput=False)
    wqt = nc.declare_dram_parameter("wqt", [D, 512], F32, isOutput=False)
    wkt = nc.declare_dram_parameter("wkt", [D, 512], F32, isOutput=False)
    wvt = nc.declare_dram_parameter("wvt", [D, 512], F32, isOutput=False)
    wot = nc.declare_dram_parameter("wot", [512, D], F32, isOutput=False)
    cost = nc.declare_dram_parameter("cost", [128, S], F32, isOutput=False)
    sint2 = nc.declare_dram_parameter("sint2", [128, S], F32, isOutput=False)
    esel2 = nc.declare_dram_parameter("esel2", [8, 4, 128], F32, isOutput=False)
    y = nc.declare_dram_parameter("y", [S, D], F32, isOutput=True)

    swap_mask = _pair_swap_mask()

    with tile.TileContext(nc) as tc, ExitStack() as ctx:
        singles = ctx.enter_context(tc.tile_pool(name="singles", bufs=1))
        ident = singles.tile([128, 128], F32)
        make_identity(nc, ident)

        # persistent tensors
        xtc = singles.tile([128, 8, S], BF16, tag="xtc")       # x^T, bf16
        qslabs = [singles.tile([128, S], F32R, tag=f"qsl{p}", name=f"qsl{p}")
                  for p in range(NP)]                          # doubles as attn out
        vslab = singles.tile([128, NKT, 8, 65], BF16, tag="vslab")
        cosc = singles.tile([128, S], F32, tag="cosc")
        sinc = singles.tile([128, S], F32, tag="sinc")
        wvr = singles.tile([128, 8, 512], BF16, tag="wvr")
        wor = singles.tile([128, 4, D], F32R, tag="wor")
        selr = singles.tile([8, 4, 128], F32R, tag="selr")
        sums = [singles.tile([8, 512], F32, tag=f"sums{p}", name=f"sums{p}")
                for p in range(NP)]

        # pools
        xpool = ctx.enter_context(tc.tile_pool(name="xpool", bufs=4))
        wst = ctx.enter_context(tc.tile_pool(name="wst", bufs=2))
        wrq = ctx.enter_context(tc.tile_pool(name="wrq", bufs=2))
        wrk = ctx.enter_context(tc.tile_pool(name="wrk", bufs=2))
        kpool = ctx.enter_context(tc.tile_pool(name="kpool", bufs=2))
        ptpool = ctx.enter_context(tc.tile_pool(name="ptpool", bufs=4))
        tshp = ctx.enter_context(tc.tile_pool(name="tshp", bufs=2))
        nrm = ctx.enter_context(tc.tile_pool(name="nrm", bufs=1))
        ysb = ctx.enter_context(tc.tile_pool(name="ysb", bufs=2))
        ppp = ctx.enter_context(tc.tile_pool(name="ppp", bufs=2, space="PSUM"))
        pss = ctx.enter_context(tc.tile_pool(name="pss", bufs=2, space="PSUM"))
        posp = ctx.enter_context(tc.tile_pool(name="posp", bufs=1, space="PSUM"))

        nc.sync.dma_start(out=cosc, in_=cost[:, :])
        nc.sync.dma_start(out=sinc, in_=sint2[:, :])

        # ---- weight staging helpers ------------------------------------
        def stage_wv():
            wv_r = wvt.rearrange("(ic p) o -> p ic o", p=128)
            for ic in range(8):
                st = wst.tile([128, 1024], F32, tag="wst", name=f"wv{ic}")
                nc.sync.dma_start(out=st[:, 0:512], in_=wv_r[:, ic, :])
                nc.vector.tensor_copy(wvr[:, ic, :], st[:, 0:512])

        def stage_wqk(p):
            """DMA + cast pair p's q/k weight slices -> [128, 8, 128] bf16."""
            tiles = {}
            for name, src, pool in (("q", wqt, wrq), ("k", wkt, wrk)):
                src_r = src.rearrange("(ic pp) o -> pp ic o", pp=128)
                st = wst.tile([128, 1024], F32, tag="wst", name=f"w{name}st{p}")
                st_v = st.rearrange("pp (ic o) -> pp ic o", ic=8)
                nc.sync.dma_start(
                    out=st_v, in_=src_r[:, :, p * 128:(p + 1) * 128])
                wr = pool.tile([128, 8, 128], BF16, tag=f"wr{name}",
                               name=f"wr{name}{p}")
                nc.vector.tensor_copy(wr, st_v)
                tiles[name] = wr
            return tiles

        def stage_wo():
            wot_r = wot.rearrange("(ic p) o -> p ic o", p=128)
            for ic in range(4):
                st = wst.tile([128, 1024], F32, tag="wst", name=f"wo{ic}")
                nc.sync.dma_start(out=st, in_=wot_r[:, ic, :])
                nc.vector.tensor_copy(wor[:, ic, :], st)

        # ---- phase helpers ---------------------------------------------
        def emit_xt_and_vproj():
            """Transpose x into xtc (bf16) and project v for all heads."""
            for sc4 in range(4):
                xts = []
                for ssub in range(4):
                    xt = xpool.tile([128, D], F32, tag="x",
                                    name=f"x{sc4}_{ssub}")
                    s0 = sc4 * 512 + ssub * 128
                    nc.sync.dma_start(out=xt, in_=xb[s0:s0 + 128, :])
                    xts.append(xt)
                for ic in range(8):
                    ptr = ppp.tile([128, 512], F32, tag="pp", name="ptr")
                    for ssub in range(4):
                        nc.tensor.transpose(
                            ptr[:, ssub * 128:(ssub + 1) * 128],
                            xts[ssub][:, ic * 128:(ic + 1) * 128], ident)
                    nc.scalar.copy(
                        out=xtc[:, ic, sc4 * 512:(sc4 + 1) * 512], in_=ptr)
                # v projection for this s-chunk (all 8 heads)
                for ssub in range(4):
                    pv = ppp.tile([128, 512], F32, tag="pp", name="pv")
                    for ic in range(8):
                        nc.tensor.matmul(
                            pv,
                            lhsT=xtc[:, ic, sc4 * 512 + ssub * 128:
                                     sc4 * 512 + (ssub + 1) * 128],
                            rhs=wvr[:, ic, :],
                            start=(ic == 0), stop=(ic == 7))
                    kt = sc4 * 4 + ssub
                    nc.vector.tensor_copy(
                        vslab[:, kt, :, 0:64],
                        pv.rearrange("p (h dk) -> p h dk", h=8))
            nc.vector.memset(vslab[:, :, :, 64:65], 1.0)

        def proj_qk_group(p, wr_tiles, t, sc4):
            """One projection group: pair p, tensor t in {q, k}, s-chunk sc4."""
            slab = qslabs[p] if t == "q" else kslab_tiles[p]
            wr = wr_tiles[t]
            ssl = slice(sc4 * 512, (sc4 + 1) * 512)
            pp = ppp.tile([128, 512], F32, tag="pp", name=f"pp{t}{p}_{sc4}")
            for ic in range(8):
                nc.tensor.matmul(
                    pp, lhsT=wr[:, ic, :], rhs=xtc[:, ic, ssl],
                    start=(ic == 0), stop=(ic == 7))
            tsh = tshp.tile([128, 512], F32, tag="tsh")
            nc.vector.stream_shuffle(tsh, pp, swap_mask)
            nc.vector.tensor_mul(tsh, tsh, sinc[:, ssl])
            nc.vector.tensor_mul(slab[:, ssl], pp, cosc[:, ssl])
            nc.vector.tensor_add(slab[:, ssl], slab[:, ssl], tsh)

        kslab_tiles = {}

        def attention_quarter(p, q, interleave):
            """Attention for pair p, q-quarter q. interleave: list of
            callables emitted spread through the j loop (projection groups of
            the next pair, or output-projection groups)."""
            qsl0 = q * 512
            kslab = kslab_tiles[p]
            qslab = qslabs[p]
            pos = posp.tile([65, 1024], F32, tag="pos", name=f"pos{p}_{q}")
            njs = 4 * q + 4
            pending = []
            il = list(interleave)
            il_every = max(1, njs // max(1, len(il)))
            for j in range(njs):
                lo = max(0, 128 * j - 512 * q)
                w = 512 - lo
                qsl = slice(qsl0 + lo, qsl0 + 512)
                # head A scores at ps[:, 0:w] (bank 0), head B at
                # ps[:, 512:512+w] (bank 1) — each matmul stays in one bank
                ps = pss.tile([128, 1024], F32, tag="ps", name=f"ps{p}_{q}_{j}")
                nc.tensor.matmul(
                    ps[:, 0:w],
                    lhsT=kslab[0:64, j * 128:(j + 1) * 128],
                    rhs=qslab[0:64, qsl], start=True, stop=True)
                nc.tensor.matmul(
                    ps[:, 512:512 + w],
                    lhsT=kslab[64:128, j * 128:(j + 1) * 128],
                    rhs=qslab[64:128, qsl], start=True, stop=True)
                pt = ptpool.tile([128, 1024], BF16, tag="pt")
                nc.scalar.activation(out=pt[:, 0:512 + w],
                                     in_=ps[:, 0:512 + w],
                                     func=AF.Exp, scale=0.125)
                if j >= 4 * q:  # diagonal tile: mask upper triangle
                    for h in range(2):
                        nc.gpsimd.affine_select(
                            out=pt[:, h * 512:h * 512 + 128],
                            in_=pt[:, h * 512:h * 512 + 128],
                            compare_op=mybir.AluOpType.is_ge,
                            fill=0.0, base=0,
                            pattern=[[1, 128]], channel_multiplier=-1)
                pending.append((j, lo, w, pt))
                # drain PV one step behind exp
                if len(pending) > 1:
                    emit_pv(p, q, pos, *pending.pop(0), njs)
                if il and (j % il_every == il_every - 1):
                    il.pop(0)()
            while pending:
                emit_pv(p, q, pos, *pending.pop(0), njs)
            for fn in il:
                fn()
            # ---- writeback (unnormalized; frees pos quickly) -----------
            stmp = nrm.tile([1, 1024], F32, tag="stmp")
            nc.vector.tensor_copy(stmp[0:1, 0:512], pos[64:65, 0:512])
            nc.vector.tensor_copy(stmp[0:1, 512:1024], pos[64:65, 512:1024])
            nc.sync.dma_start(out=sums[p][q:q + 1, :], in_=stmp[0:1, 0:512])
            nc.sync.dma_start(out=sums[p][4 + q:5 + q, :],
                              in_=stmp[0:1, 512:1024])
            qsl = slice(qsl0, qsl0 + 512)
            nc.vector.tensor_copy(qslab[0:64, qsl], pos[0:64, 0:512])
            nc.vector.tensor_copy(qslab[64:128, qsl], pos[0:64, 512:1024])

        def pair_normalize(p):
            """Scale pair p's attention output by the softmax reciprocals."""
            qslab = qslabs[p]
            recs = nrm.tile([8, 512], F32, tag="recs")
            nc.vector.reciprocal(recs, sums[p])
            recr = nrm.tile([8, 512], F32R, tag="recr")
            nc.vector.tensor_copy(recr, recs)
            for q in range(NQ):
                pb = ppp.tile([128, 512], F32, tag="pp", name=f"pb{p}_{q}")
                nc.tensor.matmul(pb, lhsT=selr[:, q, :], rhs=recr,
                                 start=True, stop=True)
                qsl = slice(q * 512, (q + 1) * 512)
                nc.vector.tensor_mul(qslab[:, qsl], qslab[:, qsl], pb)

        def emit_pv(p, q, pos, j, lo, w, pt, njs):
            start = (j == 0)
            stop = (j == njs - 1)
            nc.tensor.matmul(
                pos[:, lo:512], lhsT=vslab[:, j, 2 * p, 0:65],
                rhs=pt[:, 0:w], start=start, stop=stop)
            nc.tensor.matmul(
                pos[:, 512 + lo:1024], lhsT=vslab[:, j, 2 * p + 1, 0:65],
                rhs=pt[:, 512:512 + w], start=start, stop=stop)

        def outproj_group(qs):
            for oh in range(2):
                py = ppp.tile([128, 512], F32, tag="pp", name=f"py{qs}_{oh}")
                for p in range(NP):
                    nc.tensor.matmul(
                        py, lhsT=qslabs[p][:, qs * 128:(qs + 1) * 128],
                        rhs=wor[:, p, oh * 512:(oh + 1) * 512],
                        start=(p == 0), stop=(p == 3))
                yt = ysb.tile([128, 512], F32, tag="yt", name=f"yt{qs}_{oh}")
                nc.scalar.copy(out=yt, in_=py)
                nc.sync.dma_start(
                    out=y[qs * 128:(qs + 1) * 128, oh * 512:(oh + 1) * 512],
                    in_=yt)

        # ================= emission =====================================
        selst = nrm.tile([8, 4, 128], F32, tag="selst")
        nc.sync.dma_start(out=selst, in_=esel2[:])
        nc.vector.tensor_copy(selr, selst)
        stage_wv()
        wr_cur = stage_wqk(0)
        emit_xt_and_vproj()
        kslab_tiles[0] = kpool.tile([128, S], F32R, tag="ks", name="ks0")
        for sc4 in range(4):
            proj_qk_group(0, wr_cur, "q", sc4)
        for sc4 in range(4):
            proj_qk_group(0, wr_cur, "k", sc4)

        for p in range(NP):
            # build interleave work: projection groups of pair p+1, or the
            # output projection during the last pair
            il_chunks = [[] for _ in range(NQ)]
            if p + 1 < NP:
                wr_next = stage_wqk(p + 1)
                kslab_tiles[p + 1] = kpool.tile(
                    [128, S], F32R, tag="ks", name=f"ks{p+1}")
                work = [(t, sc4) for t in ("q", "k") for sc4 in range(4)]
                for i, (t, sc4) in enumerate(work):
                    pn, tt, ss = p + 1, t, sc4
                    il_chunks[min(NQ - 1, i * NQ // len(work))].append(
                        (lambda pn=pn, tt=tt, ss=ss, wn=wr_next:
                         proj_qk_group(pn, wn, tt, ss)))
                if p == 2:
                    il_chunks[NQ - 1].append(stage_wo)

            for q in range(NQ):
                attention_quarter(p, q, il_chunks[q])
            pair_normalize(p)

        for qs in range(16):
            outproj_group(qs)

    _split_waits(nc)
    return nc


def _host_inputs(x, wq, wk, wv, wo, token_positions):
    pos = np.asarray(token_positions).astype(np.float64)
    ex = np.arange(0, DK, 2, dtype=np.float64) / DK
    freq = 1.0 / (THETA ** ex)
    f = pos[:, None] * freq[None, :]                       # [S, DK/2]
    cos = np.repeat(np.cos(f), 2, axis=1).astype(np.float32)   # [S, DK]
    sin = np.repeat(np.sin(f), 2, axis=1).astype(np.float32)
    cosT = np.ascontiguousarray(cos.T)                     # [DK, S]
    sinT = np.ascontiguousarray(sin.T)
    sgn = np.where(np.arange(DK) % 2 == 0, -1.0, 1.0).astype(np.float32)
    sinT2 = sinT * sgn[:, None]
    costile = np.tile(cosT, (2, 1))                        # [128, S]
    sintile = np.tile(sinT2, (2, 1))

    wqT = np.ascontiguousarray(wq.T)
    wkT = np.ascontiguousarray(wk.T)
    wvT = np.ascontiguousarray(wv.T)
    woT = np.ascontiguousarray(wo.T)

    # selector for the per-(quarter) reciprocal broadcast matmul:
    # pb[m] for quarter q picks sums row q (m<64, head A) or 4+q (head B)
    esel2 = np.zeros((8, 4, 128), np.float32)
    for q in range(4):
        esel2[q, q, 0:64] = 1.0
        esel2[4 + q, q, 64:128] = 1.0

    in_maps = []
    for core in range(N_CORES):
        b, hh = core // 2, core % 2
        osl = slice(hh * 512, (hh + 1) * 512)
        in_maps.append({
            "xb": np.ascontiguousarray(x[b]),
            "wqt": np.ascontiguousarray(wqT[:, osl]),
            "wkt": np.ascontiguousarray(wkT[:, osl]),
            "wvt": np.ascontiguousarray(wvT[:, osl]),
            "wot": np.ascontiguousarray(woT[osl, :]),
            "cost": costile,
            "sint2": sintile,
            "esel2": esel2,
        })
    return in_maps


def run_sharded(x, wq, wk, wv, wo, token_positions, trace=False):
    from concourse.bass_utils import run_bass_kernel_spmd
    if "nc" not in _prog_cache:
        _prog_cache["nc"] = _build_program()
    nc = _prog_cache["nc"]
    in_maps = _host_inputs(x, wq, wk, wv, wo, token_positions)
    res = run_bass_kernel_spmd(nc, in_maps, list(range(N_CORES)), trace=trace)
    out = np.empty((B, S, D), np.float32)
    for b in range(B):
        out[b] = res.results[2 * b]["y"] + res.results[2 * b + 1]["y"]
    return out, res


def kernel(x, wq, wk, wv, wo, token_positions):
    x = np.asarray(x, dtype=np.float32)
    out, _ = run_sharded(
        x, np.asarray(wq, np.float32), np.asarray(wk, np.float32),
        np.asarray(wv, np.float32), np.asarray(wo, np.float32),
        np.asarray(token_positions))
    return out
